# revision 14
# baseline (speedup 1.0000x reference)
"""Trainium2 Bass kernel for nn_CardGNN (3-layer GATv2 message passing), v2.

Sharding: nodes partitioned across 8 NeuronCores (6250 nodes each, 50 blocks
of 125 destination nodes). Per-edge source features are fetched with bulk
bf16 dma_gather from node-major tables split at row 32750 (int16 indices);
destination features are expanded on the TensorEngine with one-hot matrices
(no gather); segment softmax + aggregation run as one-hot matmuls
accumulating in PSUM. Node features travel between layers channel-major via
a bf16 AllGather so the source transform h @ Wr needs no transposes.
"""
import math
import os
import numpy as np
import ml_dtypes

import concourse.bacc as bacc
import concourse.mybir as mybir
import concourse.tile as tile
from concourse.bass_utils import run_bass_kernel_spmd

F32 = mybir.dt.float32
BF16 = mybir.dt.bfloat16
I16 = mybir.dt.int16
AF = mybir.ActivationFunctionType
OP = mybir.AluOpType

N = 50000
E = 800000
IN = 128
HID = 32
HEADS = 4
CH = 32
HC = HID * HEADS  # 128
EMB = 128
NCORES = 8
NPC = N // NCORES          # 6250 nodes per core
BN = 125                   # dst nodes per block
NBLK = NPC // BN           # 50 blocks per core
NGB = N // BN              # 400 global blocks
P = 128
SPLIT = 32750              # = 262 * BN, < 2**15 so lo-table idx fits int16
V_LO = SPLIT
V_HI = N - SPLIT
NEG = 0.2
LAYERS = 3


def _wrap16(idx_flat):
    """dma_gather index layout: w[p, s] = idx[s*16+p], replicated to 128 rows."""
    w = idx_flat.reshape(-1, 16).T.astype(np.int16)
    return np.tile(w, (8, 1))


def _preprocess(edge_index):
    src = np.concatenate([np.asarray(edge_index[0]), np.arange(N)]).astype(np.int64)
    dst = np.concatenate([np.asarray(edge_index[1]), np.arange(N)]).astype(np.int64)
    order = np.argsort(dst, kind="stable")
    src = src[order]
    dst = dst[order]
    blk = dst // BN
    starts = np.searchsorted(blk, np.arange(NGB))
    ends = np.searchsorted(blk, np.arange(NGB) + 1)

    buckets = []
    maxlo = maxhi = 0
    for g in range(NGB):
        s, e = starts[g], ends[g]
        sg, dg = src[s:e], dst[s:e]
        m = sg < SPLIT
        lo = (sg[m], dg[m])
        hi = (sg[~m], dg[~m])
        buckets.append((lo, hi))
        maxlo = max(maxlo, len(lo[0]))
        maxhi = max(maxhi, len(hi[0]))
    NL = max(1, math.ceil(maxlo / P))
    NH = max(1, math.ceil(maxhi / P))
    NSUB = NL + NH

    idx_meta = np.zeros((NCORES, NBLK, P, 8 * NSUB), np.int16)
    dl_meta = np.full((NCORES, NBLK, P, NSUB), float(BN), ml_dtypes.bfloat16)
    dl_row = np.full((NCORES, NBLK, 1, NSUB * P), float(BN), ml_dtypes.bfloat16)
    # per-block bucket sizes, maxed over cores (the SPMD program is shared, so
    # num_idxs must be uniform; per-core shortfall is padded with idx 0 / dl=BN)
    cnts = np.zeros((NBLK, 2), np.int64)
    for g in range(NGB):
        c, b = divmod(g, NBLK)
        n0 = g * BN
        cnts[b, 0] = max(cnts[b, 0], len(buckets[g][0][0]))
        cnts[b, 1] = max(cnts[b, 1], len(buckets[g][1][0]))
        for (sg, dg), nsubs, off in (
            (buckets[g][0], NL, 0),
            (buckets[g][1], NH, NL),
        ):
            cnt = len(sg)
            jidx = np.zeros(nsubs * P, np.int64)
            jidx[:cnt] = sg if off == 0 else sg - SPLIT
            idx_meta[c, b, :, off * 8:(off + nsubs) * 8] = _wrap16(jidx)
            dl = np.full(nsubs * P, float(BN), np.float32)
            dl[:cnt] = (dg - n0).astype(np.float32)
            dl_meta[c, b, :, off:off + nsubs] = dl.reshape(nsubs, P).T.astype(ml_dtypes.bfloat16)
            dl_row[c, b, 0, off * P:(off + nsubs) * P] = dl.astype(ml_dtypes.bfloat16)
    return NL, NH, idx_meta, dl_meta, dl_row, cnts


def _bcast(v, rows=P):
    v = np.asarray(v, np.float32).reshape(-1)
    return np.tile(v[None, :], (rows, 1)).astype(np.float32)


def _build(NL, NH, cnts):
    NSUB = NL + NH
    SUBC = 4                     # subtiles per x_i psum chunk
    # per-block active subtile index lists (lo bucket at 0, hi bucket at NL)
    nact = []
    for b in range(NBLK):
        nl_b = max(1, math.ceil(int(cnts[b, 0]) / P))
        nh_b = max(1, math.ceil(int(cnts[b, 1]) / P))
        nact.append((nl_b, nh_b))
    nc = bacc.Bacc()

    # ---- I/O ----
    xT_ext = nc.declare_dram_parameter("xT", [IN, NPC], F32, isOutput=False)
    idxm_ext = nc.declare_dram_parameter("idxm", [NBLK, P, 8 * NSUB], I16, isOutput=False)
    dlm_ext = nc.declare_dram_parameter("dlm", [NBLK, P, NSUB], BF16, isOutput=False)
    dlr_ext = nc.declare_dram_parameter("dlr", [NBLK, 1, NSUB * P], BF16, isOutput=False)
    win_ext = nc.declare_dram_parameter("win", [IN, HID], F32, isOutput=False)
    binb_ext = nc.declare_dram_parameter("binb", [P, HID], F32, isOutput=False)
    wl_ext, wr_ext, blb_ext, brb_ext, attb_ext, bob_ext, gb_ext, beb_ext = [], [], [], [], [], [], [], []
    for i in range(LAYERS):
        ic = HID if i == 0 else HC
        wl_ext.append(nc.declare_dram_parameter(f"wl{i}", [ic, HC], F32, isOutput=False))
        wr_ext.append(nc.declare_dram_parameter(f"wr{i}", [ic, HC], F32, isOutput=False))
        blb_ext.append(nc.declare_dram_parameter(f"blb{i}", [P, HC], F32, isOutput=False))
        brb_ext.append(nc.declare_dram_parameter(f"brb{i}", [P, HC], F32, isOutput=False))
        attb_ext.append(nc.declare_dram_parameter(f"attb{i}", [P, HC], F32, isOutput=False))
        bob_ext.append(nc.declare_dram_parameter(f"bob{i}", [P, HC], F32, isOutput=False))
        gb_ext.append(nc.declare_dram_parameter(f"gb{i}", [P, HC], F32, isOutput=False))
        beb_ext.append(nc.declare_dram_parameter(f"beb{i}", [P, HC], F32, isOutput=False))
    wout_ext = nc.declare_dram_parameter("wout", [HC, EMB], F32, isOutput=False)
    boutb_ext = nc.declare_dram_parameter("boutb", [P, EMB], F32, isOutput=False)
    out_ext = nc.declare_dram_parameter("out", [NPC, EMB], F32, isOutput=True)

    with tile.TileContext(nc) as tc:
        with (
            tc.tile_pool(name="dram", bufs=1, space="DRAM") as dpool,
            tc.tile_pool(name="pers", bufs=1) as pers,
            tc.tile_pool(name="wpool", bufs=1) as wpool,
            tc.tile_pool(name="work", bufs=1) as work,
            tc.tile_pool(name="tbuf", bufs=4) as tbuf,
            tc.tile_pool(name="gbuf", bufs=2) as gbuf,
            tc.tile_pool(name="small", bufs=2) as small,
            tc.tile_pool(name="psA", bufs=2, space="PSUM") as psA,
            tc.tile_pool(name="psX", bufs=2, space="PSUM") as psX,
            tc.tile_pool(name="psB", bufs=2, space="PSUM") as psB,
        ):
            # ---- DRAM internals ----
            xr_lo = dpool.tile([V_LO, HC], BF16, tag="xr_lo")
            xr_hi = dpool.tile([V_HI, HC], BF16, tag="xr_hi")
            ag0_in = dpool.tile([HID, NPC], BF16, tag="ag0_in")
            ag0_out = dpool.tile([NCORES * HID, NPC], BF16, tag="ag0_out", addr_space="Shared")
            ag_in = [dpool.tile([HC, NPC], BF16, tag=f"ag_in{j}", name=f"ag_in{j}") for j in range(2)]
            ag_out = [dpool.tile([NCORES * HC, NPC], BF16, tag=f"ag_out{j}", name=f"ag_out{j}",
                                 addr_space="Shared") for j in range(2)]

            # ---- persistent SBUF ----
            hT_a = pers.tile([P, NPC], F32, tag="hT_a")      # node features, channel-major
            hT_b = pers.tile([P, NPC], F32, tag="hT_b")
            hT_range = [pers.tile([P, NPC], BF16, tag="hT_range0", name="hT_range0")] * 2
            xl_all = pers.tile([P, NBLK, HC], BF16, tag="xl_all")
            iota_t = pers.tile([P, NSUB, P], BF16, tag="iota")
            iotac_t = pers.tile([P, 1], F32, tag="iotac")
            id_t = pers.tile([P, P], F32, tag="ident")
            eps5_t = pers.tile([P, 1], F32, tag="eps5")

            acc_all = pers.tile([P, NBLK, HC + HEADS], F32, tag="acc_all")

            from concourse.masks import make_identity
            make_identity(nc, id_t[:])
            idbf_t = pers.tile([P, P], BF16, tag="idbf")
            nc.vector.tensor_copy(idbf_t[:], id_t[:])
            nc.vector.memset(eps5_t[:], 1e-5)
            nc.vector.memset(xl_all[:], 0.0)
            ioi_t = pers.tile([P, NSUB, P], mybir.dt.int32, tag="t_all", name="ioi_t")
            nc.gpsimd.iota(ioi_t[:], pattern=[[0, NSUB], [1, P]], base=0, channel_multiplier=0)
            nc.vector.tensor_copy(iota_t[:], ioi_t[:])
            ioc_t = pers.tile([P, 1], mybir.dt.int32, tag="ioc", name="ioc_t")
            nc.gpsimd.iota(ioc_t[:], pattern=[[0, 1]], base=0, channel_multiplier=1)
            nc.vector.tensor_copy(iotac_t[:], ioc_t[:])

            # ================= h0 = gelu(x @ W_in + b_in) =================
            xT_t = pers.tile([P, NPC], F32, tag="t_all", name="xT_t")
            nc.sync.dma_start(xT_t[:], xT_ext[:])
            win_t = wpool.tile([IN, HID], F32, tag="win")
            binb_t = wpool.tile([P, HID], F32, tag="binb")
            nc.sync.dma_start(win_t[:], win_ext[:])
            nc.sync.dma_start(binb_t[:], binb_ext[:])
            for b in range(NBLK):
                cs = slice(b * BN, (b + 1) * BN)
                ps = psA.tile([P, HC], F32, tag="mm")
                nc.tensor.matmul(ps[:BN, :HID], xT_t[:IN, cs], win_t[:], start=True, stop=True)
                h0s = work.tile([P, HID], F32, tag="h0s")
                nc.vector.tensor_tensor(out=h0s[:BN, :], in0=ps[:BN, :HID], in1=binb_t[:BN, :], op=OP.add)
                h0g = work.tile([P, HID], F32, tag="h0g")
                nc.scalar.activation(h0g[:BN, :], h0s[:BN, :], AF.Gelu)
                tp = psA.tile([HC, P], F32, tag="tp")
                nc.tensor.transpose(tp[:HID, :BN], h0g[:BN, :], id_t[:BN, :BN])
                nc.vector.tensor_copy(hT_a[:HID, cs], tp[:HID, :BN])

            # Warm the rotating gather buffers and et scratch: exact-count
            # gathers leave trailing subtiles unwritten, and downstream
            # elementwise ops read whole tiles — stale bits must be finite.
            for _w in range(3):
                xj_w = gbuf.tile([P, NSUB, HC], BF16, tag="xj", bufs=3)
                nc.vector.memset(xj_w[:], 0.0)
            et_w = work.tile([P, NSUB, HC], BF16, tag="et")
            nc.vector.memset(et_w[:], 0.0)

            nc.vector.tensor_copy(hT_range[0][:HID, :], hT_a[:HID, :])
            nc.sync.dma_start(ag0_in[:], hT_range[0][:HID, :])
            nc.gpsimd.collective_compute(
                "AllGather", OP.bypass, replica_groups=[list(range(NCORES))],
                ins=[ag0_in.opt()], outs=[ag0_out.opt()],
            )

            hT_prev, hT_new = hT_a, hT_b

            for li in range(LAYERS):
                ic = HID if li == 0 else HC
                agout = ag0_out if li == 0 else ag_out[li - 1]

                wl_t = wpool.tile([HC, HC], F32, tag="wl")
                wrb_t = wpool.tile([HC, HC], BF16, tag="wrb")
                blb_t = wpool.tile([P, HC], F32, tag="blb")
                brb_t = wpool.tile([P, HC], F32, tag="brb")
                attb_t = wpool.tile([P, HC], BF16, tag="attb")
                bob_t = wpool.tile([P, HC], F32, tag="bob")
                gb_t = wpool.tile([P, HC], F32, tag="gb")
                beb_t = wpool.tile([P, HC], F32, tag="beb")
                nc.sync.dma_start(wl_t[:ic, :], wl_ext[li][:])
                wrf_t = small.tile([HC, HC], F32, tag="wrf")
                nc.sync.dma_start(wrf_t[:ic, :], wr_ext[li][:])
                nc.vector.tensor_copy(wrb_t[:ic, :], wrf_t[:ic, :])
                nc.sync.dma_start(blb_t[:], blb_ext[li][:])
                nc.sync.dma_start(brb_t[:], brb_ext[li][:])
                attf_t = small.tile([P, HC], F32, tag="attf")
                nc.sync.dma_start(attf_t[:], attb_ext[li][:])
                nc.vector.tensor_copy(attb_t[:], attf_t[:])
                nc.sync.dma_start(bob_t[:], bob_ext[li][:])
                nc.sync.dma_start(gb_t[:], gb_ext[li][:])
                nc.sync.dma_start(beb_t[:], beb_ext[li][:])

                # ---- xl (own nodes) -> SBUF xl_all, bf16 ----
                for b in range(NBLK):
                    cs = slice(b * BN, (b + 1) * BN)
                    ps = psA.tile([P, HC], F32, tag="mm")
                    nc.tensor.matmul(ps[:BN, :], hT_prev[:ic, cs], wl_t[:ic, :], start=True, stop=True)
                    nc.vector.tensor_tensor(out=xl_all[:BN, b, :], in0=ps[:BN, :], in1=blb_t[:BN, :], op=OP.add)

                # ---- xr tables (all nodes, from allgathered bf16 h^T) ----
                TB = 5  # blocks per store DMA
                for cc in range(NCORES):
                    hTr = hT_range[cc % 2]
                    nc.sync.dma_start(hTr[:ic, :], agout[cc * ic:(cc + 1) * ic, :])
                    for b0 in range(0, NBLK, TB):
                        xrs = tbuf.tile([P, TB, HC], BF16, tag="tabs")
                        for k in range(TB):
                            b = b0 + k
                            cs = slice(b * BN, (b + 1) * BN)
                            ps = psA.tile([P, HC], F32, tag="mm")
                            nc.tensor.matmul(ps[:BN, :], hTr[:ic, cs], wrb_t[:ic, :], start=True, stop=True)
                            nc.vector.tensor_tensor(out=xrs[:BN, k, :], in0=ps[:BN, :], in1=brb_t[:BN, :], op=OP.add)
                        g0 = cc * NBLK + b0
                        r0 = g0 * BN
                        r1 = r0 + TB * BN
                        # [125, TB, 128] sbuf rows -> contiguous DRAM rows; lo/hi split is
                        # TB*BN-aligned iff SPLIT % (TB*BN) == 0 (it is: 32750 = 52*625 + 250; not aligned)
                        if r1 <= SPLIT:
                            nc.sync.dma_start(
                                xr_lo[r0:r1, :].rearrange("(t n) c -> n t c", t=TB), xrs[:BN, :, :])
                        elif r0 >= SPLIT:
                            nc.sync.dma_start(
                                xr_hi[r0 - SPLIT:r1 - SPLIT, :].rearrange("(t n) c -> n t c", t=TB),
                                xrs[:BN, :, :])
                        else:
                            kmid = (SPLIT - r0) // BN
                            nc.sync.dma_start(
                                xr_lo[r0:SPLIT, :].rearrange("(t n) c -> n t c", t=kmid),
                                xrs[:BN, :kmid, :])
                            nc.sync.dma_start(
                                xr_hi[0:r1 - SPLIT, :].rearrange("(t n) c -> n t c", t=TB - kmid),
                                xrs[:BN, kmid:, :])

                # ---- edge blocks ----
                def _post(b0, b1, li=li, bob_t=bob_t, gb_t=gb_t, beb_t=beb_t,
                          hT_prev=hT_prev, hT_new=hT_new):
                    HB = b1 - b0
                    t_ap = acc_all[:BN, b0:b1, :HC]
                    den_t = small.tile([P, NBLK, HEADS], F32, tag="den", name="den_t")
                    nc.vector.tensor_scalar(out=den_t[:BN, :HB, :], in0=acc_all[:BN, b0:b1, HC:],
                                            scalar1=1e-16, scalar2=None, op0=OP.add)
                    rec_t = small.tile([P, NBLK, HEADS], F32, tag="rec", name="rec_t")
                    nc.vector.reciprocal(rec_t[:BN, :HB, :], den_t[:BN, :HB, :])
                    nc.vector.tensor_tensor(
                        out=t_ap.rearrange("p b (h c) -> p b h c", h=HEADS),
                        in0=t_ap.rearrange("p b (h c) -> p b h c", h=HEADS),
                        in1=rec_t[:BN, :HB, :, None].broadcast_to([BN, HB, HEADS, CH]), op=OP.mult)
                    nc.vector.tensor_tensor(
                        out=t_ap, in0=t_ap,
                        in1=bob_t[:BN, None, :].broadcast_to([BN, HB, HC]), op=OP.add)
                    mu_t = small.tile([P, NBLK], F32, tag="mu", name="mu_t")
                    nc.vector.reduce_sum(mu_t[:BN, :HB], t_ap, axis=mybir.AxisListType.X)
                    nc.vector.tensor_scalar(out=mu_t[:BN, :HB], in0=mu_t[:BN, :HB],
                                            scalar1=1.0 / HC, scalar2=None, op0=OP.mult)
                    nc.vector.tensor_tensor(
                        out=t_ap, in0=t_ap,
                        in1=mu_t[:BN, :HB, None].broadcast_to([BN, HB, HC]), op=OP.subtract)
                    var_t = small.tile([P, NBLK], F32, tag="var", name="var_t")
                    sqs_t = small.tile([P, HC], F32, tag="sqs", name="sqs_t")
                    for b in range(b0, b1):
                        nc.scalar.activation(sqs_t[:BN, :], acc_all[:BN, b, :HC], AF.Square,
                                             accum_out=var_t[:BN, b - b0:b - b0 + 1])
                    std_t = small.tile([P, NBLK], F32, tag="std", name="std_t")
                    nc.scalar.activation(std_t[:BN, :HB], var_t[:BN, :HB], AF.Sqrt,
                                         scale=1.0 / HC, bias=eps5_t[:BN, :1])
                    rstd_t = small.tile([P, NBLK], F32, tag="rstd", name="rstd_t")
                    nc.vector.reciprocal(rstd_t[:BN, :HB], std_t[:BN, :HB])
                    nc.vector.tensor_tensor(
                        out=t_ap, in0=t_ap,
                        in1=rstd_t[:BN, :HB, None].broadcast_to([BN, HB, HC]), op=OP.mult)
                    nc.vector.tensor_tensor(
                        out=t_ap, in0=t_ap,
                        in1=gb_t[:BN, None, :].broadcast_to([BN, HB, HC]), op=OP.mult)
                    nc.vector.tensor_tensor(
                        out=t_ap, in0=t_ap,
                        in1=beb_t[:BN, None, :].broadcast_to([BN, HB, HC]), op=OP.add)
                    nc.scalar.activation(t_ap, t_ap, AF.Gelu)
                    for b in range(b0, b1):
                        cs = slice(b * BN, (b + 1) * BN)
                        tp = psA.tile([HC, P], F32, tag="tp")
                        nc.tensor.transpose(tp[:, :BN], acc_all[:BN, b, :HC], id_t[:BN, :BN])
                        if li == 0:
                            nc.vector.tensor_copy(hT_new[:, cs], tp[:, :BN])
                        else:
                            nc.vector.tensor_tensor(out=hT_new[:, cs], in0=tp[:, :BN],
                                                    in1=hT_prev[:, cs], op=OP.add)

                for b in range(NBLK):
                    if b == NBLK // 2 + 2:
                        _post(0, NBLK // 2)
                    nl_b, nh_b = nact[b]
                    cl = max(1, int(cnts[b, 0]))
                    ch = max(1, int(cnts[b, 1]))
                    idxm_t = small.tile([P, 8 * NSUB], I16, tag="idxm")
                    dlm_t = small.tile([P, NSUB], BF16, tag="dlm")
                    dstb_t = work.tile([P, NSUB * P], BF16, tag="dstb")
                    nc.sync.dma_start(idxm_t[:], idxm_ext[b])
                    nc.sync.dma_start(dlm_t[:], dlm_ext[b])
                    nc.sync.dma_start(dstb_t[:], dlr_ext[b, :1, :].broadcast_to([P, NSUB * P]))

                    xj_t = gbuf.tile([P, NSUB, HC], BF16, tag="xj", bufs=3)
                    nc.gpsimd.dma_gather(
                        out_ap=xj_t[:, :nl_b, :], in_ap=xr_lo[:], idxs_ap=idxm_t[:, :nl_b * 8],
                        num_idxs=cl, num_idxs_reg=cl, elem_size=HC, single_packet=False)
                    nc.gpsimd.dma_gather(
                        out_ap=xj_t[:, NL:NL + nh_b, :], in_ap=xr_hi[:],
                        idxs_ap=idxm_t[:, NL * 8:(NL + nh_b) * 8],
                        num_idxs=ch, num_idxs_reg=ch, elem_size=HC, single_packet=False)

                    st_t = gbuf.tile([P, NSUB, P], BF16, tag="st")
                    nc.vector.tensor_tensor(
                        out=st_t[:], in0=iota_t[:],
                        in1=dlm_t[:, :, None].broadcast_to([P, NSUB, P]), op=OP.is_equal)
                    s_t = work.tile([P, NSUB * P], BF16, tag="s_t")
                    nc.vector.tensor_scalar(
                        out=s_t[:], in0=dstb_t[:], scalar1=iotac_t[:, :1], scalar2=None,
                        op0=OP.is_equal)

                    # x_i expansion on PE + add gathered x_j (vector), per
                    # active-subtile chunk within each bucket.
                    et_t = work.tile([P, NSUB, HC], BF16, tag="et")
                    for joff, njb in ((0, nl_b), (NL, nh_b)):
                        for j0 in range(joff, joff + njb, SUBC):
                            cw = min(SUBC, joff + njb - j0)
                            xi_ps = psX.tile([P, SUBC * HC], F32, tag="xi")
                            for j in range(j0, j0 + cw):
                                nc.tensor.matmul(
                                    xi_ps[:, (j - j0) * HC:(j - j0 + 1) * HC],
                                    s_t[:, j * P:(j + 1) * P], xl_all[:, b, :],
                                    start=True, stop=True)
                            nc.vector.tensor_tensor(
                                out=et_t[:, j0:j0 + cw, :],
                                in0=xi_ps[:, :cw * HC].rearrange("p (j c) -> p j c", c=HC),
                                in1=xj_t[:, j0:j0 + cw, :], op=OP.add)
                    nc.scalar.activation(et_t[:], et_t[:], AF.Prelu, alpha=NEG)
                    nc.vector.tensor_tensor(
                        out=et_t[:], in0=et_t[:],
                        in1=attb_t[:, None, :].broadcast_to([P, NSUB, HC]), op=OP.mult)
                    # per-head sum over 32 channels as a bf16 halving tree
                    # (tensor_tensor runs 2x mode; tensor_reduce is stuck at 1x)
                    etv = et_t[:].rearrange("p j (h c) -> p j h c", h=HEADS)
                    tr1 = work.tile([P, NSUB, HEADS, 16], BF16, tag="tr1")
                    nc.vector.tensor_tensor(out=tr1[:], in0=etv[:, :, :, :16],
                                            in1=etv[:, :, :, 16:], op=OP.add)
                    tr2 = work.tile([P, NSUB, HEADS, 8], BF16, tag="tr2")
                    nc.vector.tensor_tensor(out=tr2[:], in0=tr1[:, :, :, :8],
                                            in1=tr1[:, :, :, 8:], op=OP.add)
                    tr3 = work.tile([P, NSUB, HEADS, 4], BF16, tag="tr3")
                    nc.vector.tensor_tensor(out=tr3[:], in0=tr2[:, :, :, :4],
                                            in1=tr2[:, :, :, 4:], op=OP.add)
                    tr4 = work.tile([P, NSUB, HEADS, 2], BF16, tag="tr4")
                    nc.vector.tensor_tensor(out=tr4[:], in0=tr3[:, :, :, :2],
                                            in1=tr3[:, :, :, 2:], op=OP.add)
                    lg_t = small.tile([P, NSUB, HEADS], F32, tag="lg")
                    nc.vector.tensor_tensor(out=lg_t[:], in0=tr4[:, :, :, 0],
                                            in1=tr4[:, :, :, 1], op=OP.add)
                    ex_t = small.tile([P, NSUB, HEADS], BF16, tag="ex")
                    nc.scalar.activation(ex_t[:], lg_t[:], AF.Exp)

                    v_t = work.tile([P, NSUB, HC + HEADS], BF16, tag="v")
                    nc.vector.tensor_tensor(
                        out=v_t[:, :, :HC].rearrange("p j (h c) -> p j h c", h=HEADS),
                        in0=xj_t[:].rearrange("p j (h c) -> p j h c", h=HEADS),
                        in1=ex_t[:, :, :, None].broadcast_to([P, NSUB, HEADS, CH]), op=OP.mult)
                    nc.vector.tensor_copy(v_t[:, :, HC:], ex_t[:])

                    acc = psB.tile([P, HC + HEADS], F32, tag="acc")
                    active = list(range(nl_b)) + list(range(NL, NL + nh_b))
                    for k, j in enumerate(active):
                        nc.tensor.matmul(acc[:], st_t[:, j, :], v_t[:, j, :],
                                         start=(k == 0), stop=(k == len(active) - 1))

                    nc.vector.tensor_copy(acc_all[:BN, b, :], acc[:BN, :])


                _post(NBLK // 2, NBLK)

                hT_prev, hT_new = hT_new, hT_prev
                if li < LAYERS - 1:
                    nc.vector.tensor_copy(hT_range[0][:], hT_prev[:])
                    nc.sync.dma_start(ag_in[li][:], hT_range[0][:])
                    nc.gpsimd.collective_compute(
                        "AllGather", OP.bypass, replica_groups=[list(range(NCORES))],
                        ins=[ag_in[li].opt()], outs=[ag_out[li].opt()],
                    )

            # ================= out = normalize(h @ W_out + b_out) =================
            wout_t = wpool.tile([HC, EMB], F32, tag="wout")
            boutb_t = wpool.tile([P, EMB], F32, tag="boutb")
            nc.sync.dma_start(wout_t[:], wout_ext[:])
            nc.sync.dma_start(boutb_t[:], boutb_ext[:])
            for b in range(NBLK):
                cs = slice(b * BN, (b + 1) * BN)
                ps = psA.tile([P, EMB], F32, tag="mm")
                nc.tensor.matmul(ps[:BN, :], hT_prev[:HC, cs], wout_t[:], start=True, stop=True)
                osb = work.tile([P, EMB], F32, tag="osb")
                nc.vector.tensor_tensor(out=osb[:BN, :], in0=ps[:BN, :], in1=boutb_t[:BN, :], op=OP.add)
                sq_t = work.tile([P, EMB], F32, tag="osq")
                nsq_t = small.tile([P, 1], F32, tag="nsq")
                nc.scalar.activation(sq_t[:BN, :], osb[:BN, :], AF.Square, accum_out=nsq_t[:BN, :1])
                nrm_t = small.tile([P, 1], F32, tag="nrm")
                nc.scalar.activation(nrm_t[:BN, :], nsq_t[:BN, :], AF.Sqrt)
                nc.vector.tensor_scalar(out=nrm_t[:BN, :], in0=nrm_t[:BN, :], scalar1=1e-12,
                                        scalar2=None, op0=OP.max)
                recn_t = small.tile([P, 1], F32, tag="recn")
                nc.vector.reciprocal(recn_t[:BN, :], nrm_t[:BN, :])
                nc.vector.tensor_scalar(out=osb[:BN, :], in0=osb[:BN, :], scalar1=recn_t[:BN, :1],
                                        scalar2=None, op0=OP.mult)
                nc.sync.dma_start(out_ext[cs, :], osb[:BN, :])

    nc.compile()
    return nc


def _make_in_maps(inputs, idx_meta, dl_meta, dl_row):
    x = np.asarray(inputs["x"], np.float32)
    common = {
        "win": np.asarray(inputs["W_in"], np.float32),
        "binb": _bcast(inputs["b_in"]),
        "wout": np.asarray(inputs["W_out"], np.float32),
        "boutb": _bcast(inputs["b_out"]),
    }
    for i in range(LAYERS):
        common[f"wl{i}"] = np.asarray(inputs[f"Wl{i}"], np.float32)
        common[f"wr{i}"] = np.asarray(inputs[f"Wr{i}"], np.float32)
        common[f"blb{i}"] = _bcast(inputs[f"bl{i}"])
        common[f"brb{i}"] = _bcast(inputs[f"br{i}"])
        common[f"attb{i}"] = _bcast(np.asarray(inputs[f"att{i}"], np.float32).reshape(-1))
        common[f"bob{i}"] = _bcast(inputs[f"bo{i}"])
        common[f"gb{i}"] = _bcast(inputs[f"g{i}"])
        common[f"beb{i}"] = _bcast(inputs[f"be{i}"])
    in_maps = []
    for c in range(NCORES):
        m = dict(common)
        m["xT"] = np.ascontiguousarray(x[c * NPC:(c + 1) * NPC, :].T)
        m["idxm"] = idx_meta[c]
        m["dlm"] = dl_meta[c]
        m["dlr"] = dl_row[c]
        in_maps.append(m)
    return in_maps


def kernel(**inputs):
    edge_index = np.asarray(inputs["edge_index"])
    NL, NH, idx_meta, dl_meta, dl_row, cnts = _preprocess(edge_index)
    nc = _build(NL, NH, cnts)
    in_maps = _make_in_maps(inputs, idx_meta, dl_meta, dl_row)
    res = run_bass_kernel_spmd(nc, in_maps, list(range(NCORES)))
    out = np.concatenate([res.results[c]["out"] for c in range(NCORES)], axis=0)
    return out.astype(np.float32)

